# revision 51
# baseline (speedup 1.0000x reference)
"""Multi-head graph attention (GAT-style) Trainium2 Bass kernel.

Full-input contract: kernel(**inputs) takes the complete arrays, shards
batch-wise across 8 NeuronCores (2 batches each), and gathers the output.

Math per batch b, head h (KD=16 head dim):
  Q = h @ Wq_h, K = h @ Wk_h, V = h @ Wv_h            [N, 16]
  compatT[m, n] = (K Q^T)[m, n]                        [N, N] (transposed)
  p = exp(0.25 * compatT) * adjT                       (mask after exp; exact:
      masked entries are exactly 0, matching softmax(-inf) * adj)
  headsT[v, n] = (V'.T @ p)  with V' = [V | 1]         -> row 16 = denominator
  out[n, :] = sum_h (headsT_h / denom_h).T @ Wout_h + h[n, :]

The scalar engine's 128 exp instructions (~1.1us each over [128,1024] PSUM
tiles) are the hard roofline (~140us busy; ACT runs 1 elem/cycle/lane at
1.2GHz, no other engine has exp); everything else is scheduled to hide
under it:
  - hT / weights shipped bf16 from host; all-bf16 projections and PV.
  - heads regrouped as {0,2,4,6} (group 0) / {1,3,5,7} (group 1): heads
    0/2/4 are read by compat straight from the packed projection tile at
    partition bases 0/32/64 via tile_position rows (no shift DMAs on the
    first-compat critical path); the other five shift to a [16,5,N] tile
    during group 0. headsTu/den/Wout use slot order (Wout rows reordered
    host-side).
  - per-mc emission order [c0 c1 PV(prev-mc) c2 c3]: each compat reuses the
    PSUM buffer freed by the exp two slots earlier, and the PV quad (4
    heads column-packed via tile_position, which overlaps their streams)
    sits where the PE would otherwise idle.
  - V projection spread one chunk per mc-yield; batch b+1's load + QK
    projection emitted at batch-b group-1 start, its V chunks across
    group-1's yields; batch b's normalize/out-project deferred into batch
    b+1's group-0 stream so the reciprocal DMA chain is off the critical
    path.
  - DMA descriptors cost ~600ns each to issue and one descriptor rides one
    HW queue, so bulk loads go on the gpsimd DGE queue, hT split by column
    halves, adj split per m-chunk, group-end row unpacks split across
    sync+gpsimd.
  - denominator reciprocal on [128, 64] (n spread over 16 partitions via
    reshape DMA) because InstReciprocal costs free-size x 8 passes; the
    per-(head,n) reciprocal broadcast to all 16 v-rows is a single
    DRAM-bounce DMA with a stride-0 AP dimension.

Measured no-win dead ends: fp8 DoubleRow compat (HW streams the doubled
ifmap serially, and fp8 Q/K alone costs 3e-2 rel err), 1024-col matmuls
(s3d3_mm_num_elements), exp-pair [128,2048] tiles (PSUM bank budget),
custom-DVE exp (no float<->int convert; shifts return 0), splitting PV
quads (breaks the column-pack overlap), split-accumulated finish across
tile_position row bases (wrong results).
"""

import numpy as np
import ml_dtypes
from contextlib import ExitStack

import concourse.bass as bass
import concourse.mybir as mybir
import concourse.tile as tile
from concourse.bass_utils import run_bass_kernel_spmd

B, N, E, H, KD = 16, 1024, 128, 8, 16
CORES = 8
BPC = B // CORES  # batches per core
F32 = mybir.dt.float32
BF16 = mybir.dt.bfloat16
NT = 512  # PSUM bank = 512 fp32 -> max matmul out free dim
MC = N // 128  # number of 128-row chunks of m / n


def build_kernel():
    nc = bass.Bass()
    hT_d = nc.dram_tensor("ht", [BPC, E, N], BF16, kind="ExternalInput")
    h_d = nc.dram_tensor("hn", [BPC, N, E], F32, kind="ExternalInput")
    adjt_d = nc.dram_tensor("adjt", [BPC, N, N], BF16, kind="ExternalInput")
    wq_d = nc.dram_tensor("wq", [E, H * KD], BF16, kind="ExternalInput")
    wk_d = nc.dram_tensor("wk", [E, H * KD], BF16, kind="ExternalInput")
    wv_d = nc.dram_tensor("wv", [E, H * KD], BF16, kind="ExternalInput")
    wo_d = nc.dram_tensor("wo", [H * KD, E], BF16, kind="ExternalInput")
    out_d = nc.dram_tensor("out", [BPC, N, E], F32, kind="ExternalOutput")

    with ExitStack() as ctx:
        tc = ctx.enter_context(tile.TileContext(nc))
        consts = ctx.enter_context(tc.tile_pool(name="consts", bufs=1))
        io_pool = ctx.enter_context(tc.tile_pool(name="io", bufs=2))
        qk_pool = ctx.enter_context(tc.tile_pool(name="qk", bufs=2))
        v_pool = ctx.enter_context(tc.tile_pool(name="v", bufs=2))
        pt_pool = ctx.enter_context(tc.tile_pool(name="pt", bufs=8))
        p_pool = ctx.enter_context(tc.tile_pool(name="p", bufs=3))
        hd_pool = ctx.enter_context(tc.tile_pool(name="hd", bufs=2))
        ob_pool = ctx.enter_context(tc.tile_pool(name="ob", bufs=2))
        # PSUM: 8 banks of [128, 512]f32. compat 2x2 + PV 2 + misc 2 = 8.
        ps_c = ctx.enter_context(tc.tile_pool(name="ps_c", bufs=2, space="PSUM"))
        ps_h = ctx.enter_context(tc.tile_pool(name="ps_h", bufs=1, space="PSUM"))
        ps_m = ctx.enter_context(tc.tile_pool(name="ps_m", bufs=1, space="PSUM"))
        dram = ctx.enter_context(tc.tile_pool(name="dram", bufs=2, space="DRAM"))

        wq_sb = consts.tile([E, H * KD], BF16, tag="wq")
        wk_sb = consts.tile([E, H * KD], BF16, tag="wk")
        wv_sb = consts.tile([E, H * KD], BF16, tag="wv")
        wo_sb = consts.tile([H * KD, E], BF16, tag="wo")
        # weights on gpsimd so the sync queue's first descriptors are the
        # hT halves that gate the first projection matmul
        nc.gpsimd.dma_start(out=wq_sb, in_=wq_d[:, :])
        nc.gpsimd.dma_start(out=wk_sb, in_=wk_d[:, :])
        nc.gpsimd.dma_start(out=wv_sb, in_=wv_d[:, :])
        nc.gpsimd.dma_start(out=wo_sb, in_=wo_d[:, :])

        def load(b):
            # bulk loads go on the gpsimd DGE queue: the sync sequencer's
            # ~600ns per-DMA issue otherwise serializes ahead of the Q/K
            # shift DMAs that gate the first compat matmul
            # one descriptor = one HW queue, so split hT by column halves:
            # the first QK projection matmul only needs cols 0..511
            hT_sb = io_pool.tile([E, N], BF16, tag="ht")
            nc.sync.dma_start(out=hT_sb[:, 0:NT], in_=hT_d[b, :, 0:NT])
            nc.sync.dma_start(out=hT_sb[:, NT:N], in_=hT_d[b, :, NT:N])
            adjT_sb = io_pool.tile([128, MC, N], BF16, tag="adj")
            # split per m-chunk so mask(mc0) doesn't wait on the full 2MB
            for m in range(MC):
                nc.gpsimd.dma_start(
                    out=adjT_sb[:, m, :], in_=adjt_d[b, m * 128 : (m + 1) * 128, :]
                )
            h_sb = io_pool.tile([128, MC, E], F32, tag="hn")
            nc.gpsimd.dma_start(
                out=h_sb, in_=h_d[b].rearrange("(c p) e -> p c e", p=128)
            )
            return hT_sb, h_sb, adjT_sb

        def proj_qk(hT_sb):
            """Q/K projections (bf16). Group-0 heads {0,2,4} are read by
            compat directly from the packed tile at partition bases
            0/32/64 (legal tile_position rows) -> zero shift DMAs on the
            first-compat critical path. Heads {6,1,3,5,7} are shifted to a
            [16, 5, N] tile during group 0 (5 descriptors per tensor)."""
            packs = []
            for w_sb, tag in ((wq_sb, "q"), (wk_sb, "k")):
                ps = ps_m.tile([128, N], F32, tag="misc")
                for nt in range(N // NT):
                    nc.tensor.matmul(
                        out=ps[:, nt * NT : (nt + 1) * NT],
                        lhsT=w_sb,
                        rhs=hT_sb[:, nt * NT : (nt + 1) * NT],
                        start=True,
                        stop=True,
                    )
                packed = qk_pool.tile([128, N], BF16, tag=f"{tag}pk")
                nc.vector.tensor_copy(out=packed, in_=ps)
                per16 = qk_pool.tile([16, 5, N], BF16, tag=f"{tag}16")
                packs.append((packed, per16))
            # head 6 would need base partition 96 (only 0/32/64 are legal),
            # so it is shifted along with the odd heads
            for idx, hi in enumerate((6, 1, 3, 5, 7)):
                for packed, per16 in packs:
                    nc.sync.dma_start(
                        out=per16[:, idx, :],
                        in_=packed[hi * KD : (hi + 1) * KD, :],
                    )
            return packs

        def proj_v_chunk(b, m):
            """One V chunk: natural [m, h, 17] bf16, col 16 = ones."""
            st = state[b]
            v_ps = ps_m.tile([128, N], F32, tag="misc")
            nc.tensor.matmul(
                out=v_ps[:, 0 : H * KD],
                lhsT=st["hT"][:, m * 128 : (m + 1) * 128],
                rhs=wv_sb,
                start=True,
                stop=True,
            )
            vt = v_pool.tile([128, H, KD + 1], BF16, tag=f"v{m}")
            nc.vector.tensor_copy(
                out=vt[:, :, 0:KD],
                in_=v_ps[:, 0 : H * KD].rearrange("p (h k) -> p h k", k=KD),
            )
            nc.vector.memset(vt[:, :, KD : KD + 1], 1.0)
            st["v"][m] = vt

        state = [None] * BPC

        def start_batch(b):
            hT_sb, h_sb, adjT_sb = load(b)
            qpk, kpk = proj_qk(hT_sb)
            headsTu = hd_pool.tile([128, N], F32, tag="hTu", name="headsTu")
            den128 = hd_pool.tile([128, N // 16], F32, tag="den128", name="den128")
            state[b] = dict(
                hT=hT_sb, h=h_sb, adjT=adjT_sb, qpk=qpk, kpk=kpk,
                v=[None] * MC, headsTu=headsTu, den128=den128,
            )

        HEADS = ((0, 2, 4, 6), (1, 3, 5, 7))

        def attn_group(b, g):
            """Attention for heads HEADS[g] of batch b (slot = 4g+jj).
            Yields after each mc's emission so callers can interleave
            projection work."""
            st = state[b]
            adjT_sb = st["adjT"]
            hp4 = ps_h.tile([128, N], F32, tag="heads")  # slot j at rows 32j

            def emit_compat(m, jj):
                hi = HEADS[g][jj]
                if g == 0 and jj < 3:
                    # heads 0/2/4: direct from the packed tile at partition
                    # base hi*16 in {0,32,64} (legal tile_position rows)
                    k_ap = st["kpk"][0][hi * KD : (hi + 1) * KD, :]
                    q_ap = st["qpk"][0][hi * KD : (hi + 1) * KD, :]
                else:
                    idx = 0 if g == 0 else 1 + jj  # per16 slot
                    k_ap = st["kpk"][1][:, idx, :]
                    q_ap = st["qpk"][1][:, idx, :]
                c_ps = ps_c.tile([128, N], F32, tag="compat")
                for nt in range(N // NT):
                    nc.tensor.matmul(
                        out=c_ps[:, nt * NT : (nt + 1) * NT],
                        lhsT=k_ap[:, m * 128 : (m + 1) * 128],
                        rhs=q_ap[:, nt * NT : (nt + 1) * NT],
                        start=True,
                        stop=True,
                    )
                pT = pt_pool.tile([128, N], BF16, tag="pt")
                nc.scalar.activation(
                    out=pT,
                    in_=c_ps,
                    func=mybir.ActivationFunctionType.Exp,
                    scale=0.25,
                )
                pm = p_pool.tile([128, N], BF16, tag=f"pm{jj}")
                nc.vector.tensor_mul(pm, pT, adjT_sb[:, m, :])
                return pm

            def emit_pv(pms, m, jjs):
                vt = st["v"][m]
                for nt in range(N // NT):
                    for jj in jjs:
                        nc.tensor.matmul(
                            out=hp4[
                                32 * jj : 32 * jj + KD + 1,
                                nt * NT : (nt + 1) * NT,
                            ],
                            lhsT=vt[:, HEADS[g][jj], :],
                            rhs=pms[jj][:, nt * NT : (nt + 1) * NT],
                            start=(m == 0),
                            stop=(m == MC - 1),
                            tile_position=(0, 32 * jj),
                        )

            prev = None
            for m in range(MC):
                cur = [emit_compat(m, 0), emit_compat(m, 1)]
                if prev is not None:
                    emit_pv(prev, m - 1, (0, 1, 2, 3))
                cur += [emit_compat(m, 2), emit_compat(m, 3)]
                prev = cur
                yield m
            emit_pv(prev, MC - 1, (0, 1, 2, 3))
            # unpack heads + denominator rows: PSUM->SBUF copy on the idle
            # Pool engine (DMA can't read PSUM; DVE is near its budget),
            # then row DMAs on the tensor DGE queue (sync queue is the
            # startup hot path)
            hu4 = hd_pool.tile([128, N], F32, tag="huh")
            if b == BPC - 1 and g == 1:
                # tail-exposed copy: scalar engine is idle after the last exp
                nc.scalar.activation(
                    out=hu4, in_=hp4, func=mybir.ActivationFunctionType.Copy
                )
            else:
                nc.vector.tensor_copy(out=hu4, in_=hp4)
            # den rows first (they gate the reciprocal -> finish chain) on
            # sync; bulkier headsTu rows in parallel on gpsimd
            for jj in range(4):
                hi = g * 4 + jj  # slot index (wo is slot-reordered host-side)
                # [1, 1024] row -> [16, 64] block (reshape across partitions;
                # dst has real partition strides, so legal)
                nc.sync.dma_start(
                    out=st["den128"][hi * 16 : (hi + 1) * 16, :],
                    in_=hu4[32 * jj + KD : 32 * jj + KD + 1, :],
                )
            for jj in range(4):
                hi = g * 4 + jj
                nc.gpsimd.dma_start(
                    out=st["headsTu"][hi * KD : (hi + 1) * KD, :],
                    in_=hu4[32 * jj : 32 * jj + KD, :],
                )

        def recip_path(b):
            # den [8, N] lives as [128, N/16] (n spread over 16 partitions)
            # because InstReciprocal cost is free-size * 8 iterative passes
            st = state[b]
            recip128 = hd_pool.tile([128, N // 16], F32, tag="rec128")
            nc.vector.reciprocal(out=recip128, in_=st["den128"])
            rec_dram = dram.tile([8, N], F32, tag="recd")
            nc.gpsimd.dma_start(
                out=rec_dram.rearrange("h (a b) -> (h a) b", a=16),
                in_=recip128,
            )
            recip_bc = hd_pool.tile([128, N], F32, tag="recbc")
            bc = bass.AP(
                tensor=rec_dram.tensor,
                offset=rec_dram.offset,
                ap=[[N, H], [0, KD], [1, N]],
            )
            nc.gpsimd.dma_start(out=recip_bc, in_=bc)
            st["recip_bc"] = recip_bc

        def finish(b, last=False):
            st = state[b]
            headsTn = hd_pool.tile([128, N], BF16, tag="hnorm")
            nc.vector.tensor_mul(headsTn, st["headsTu"], st["recip_bc"])
            for cc in range(MC):
                # in the exposed tail (last batch) alternate PSUM pools so
                # the 8 out-projections pipeline instead of serializing on
                # one buffer; mid-stream ps_c belongs to the compat pipeline
                pool = ps_c if (last and cc % 2) else ps_m
                o_ps = pool.tile(
                    [128, N], F32, tag="compat" if pool is ps_c else "misc"
                )
                nc.tensor.matmul(
                    out=o_ps[:, 0:E],
                    lhsT=headsTn[:, cc * 128 : (cc + 1) * 128],
                    rhs=wo_sb,
                    start=True,
                    stop=True,
                )
                ob = ob_pool.tile([128, E], F32, tag="ob")
                nc.vector.tensor_add(ob, o_ps[:, 0:E], st["h"][:, cc, :])
                nc.sync.dma_start(
                    out=out_d[b, cc * 128 : (cc + 1) * 128, :], in_=ob
                )

        # ---- pipelined emission ----
        start_batch(0)
        pending = None
        for b in range(BPC):
            for g in range(2):
                for m in attn_group(b, g):
                    if g == 0:
                        # spread own V chunks under group-0's exp stream
                        # (batch 1's were already built during batch 0's g1)
                        if state[b]["v"][m] is None:
                            proj_v_chunk(b, m)
                        if m == 2 and pending is not None:
                            finish(pending)
                            pending = None
                    else:
                        if b + 1 < BPC:
                            if m == 0:
                                start_batch(b + 1)
                            proj_v_chunk(b + 1, m)
            recip_path(b)
            pending = b
        finish(pending, last=True)
    return nc


def _split_multi_waits(nc):
    """walrus codegen in this container allows only one sync-wait per
    instruction; hoist extra waits onto preceding same-engine nops."""
    import copy
    import bass_rust

    tmpl_nc = bass.Bass()
    tmpls = {}
    for en in ["vector", "scalar", "tensor", "gpsimd", "sync"]:
        ins = getattr(tmpl_nc, en).nop().ins
        tmpls[str(ins.engine)] = ins

    uid = [0]
    for fn in nc.m.functions:
        for bb in fn.blocks:
            out = []
            for ins in bb.instructions:
                si = ins.sync_info
                waits = list(si.on_wait) if si is not None else []
                if len(waits) > 1:
                    for w in waits[:-1]:
                        nop = copy.deepcopy(tmpls[str(ins.engine)])
                        uid[0] += 1
                        nop.name = f"I-splitw-{uid[0]}"
                        nop.sync_info = bass_rust.SyncInfo(
                            on_wait=[w], on_update=[]
                        )
                        out.append(nop)
                    ins.sync_info = bass_rust.SyncInfo(
                        on_wait=[waits[-1]], on_update=list(si.on_update)
                    )
                out.append(ins)
            bb.instructions = out
    return nc


def _optimize_act_waits(nc):
    """The ACT sequencer has no instruction lookahead, so each of its
    instructions costs ~100ns of decode+wait processing. Steady state per
    exp is [nop(wait ACT-self), nop(wait PE), exp(wait DVE)]:
    (1) drop ACT nops waiting on ACT's own semaphore — that WAW (exp i vs
        exp i-8 reusing a pT buffer) is already ordered by in-order
        same-engine execution;
    (2) in runs of [nop(PE), exp(DVE)] pairs, put the PE compat RAW wait
        (the tight one) on the exp itself and hoist one DVE wait — the
        strongest (last) of up to 4 pairs — in front of the run. The DVE
        wait guards pT-buffer reuse vs mask(i-8); with the 8-deep pT pool
        and masks trailing exps by ~1 slot it is satisfied ~4 exp periods
        before it is checked, so hoisting cannot stall."""
    import bass_rust

    def is_act(x):
        return "Activation" in str(x.engine)

    def waits(x):
        return list(x.sync_info.on_wait) if x.sync_info else []

    def upds(x):
        return list(x.sync_info.on_update) if x.sync_info else []

    for fn in nc.m.functions:
        for bb in fn.blocks:
            # pass 1: drop redundant ACT-self-wait nops
            kept = []
            for ins in bb.instructions:
                if (
                    is_act(ins)
                    and type(ins).__name__ == "InstNoOp"
                    and not upds(ins)
                ):
                    w = waits(ins)
                    if len(w) == 1 and str(w[0].ant_name).startswith(
                        "Activation"
                    ):
                        continue
                kept.append(ins)
            # pass 2: on the ACT-only subsequence (other engines' entries
            # interleave in the merged list), merge [nop(PE), exp(DVE)]
            # pairs in groups of 4: exp takes its nop's PE wait; the first
            # nop takes the last pair's DVE wait; the other nops are
            # deleted. Instructions are not reordered.
            act_idx = [k for k, x in enumerate(kept) if is_act(x)]
            drop = set()
            p = 0
            while p < len(act_idx):
                run = []
                q = p
                while q + 1 < len(act_idx) and len(run) < 4:
                    if act_idx[q + 1] != act_idx[q] + 1 and any(
                        is_act(kept[t])
                        for t in range(act_idx[q] + 1, act_idx[q + 1])
                    ):
                        break  # unreachable; adjacency in ACT order holds
                    a, b = kept[act_idx[q]], kept[act_idx[q + 1]]
                    if not (
                        type(a).__name__ == "InstNoOp"
                        and type(b).__name__ == "InstActivation"
                    ):
                        break
                    wa, wb = waits(a), waits(b)
                    if (
                        len(wa) != 1
                        or len(wb) != 1
                        or upds(a)
                        or not str(wa[0].ant_name).startswith("PE")
                        or not str(wb[0].ant_name).startswith("DVE")
                    ):
                        break
                    run.append((a, b))
                    q += 2
                if len(run) >= 2:
                    pe_waits = [waits(a)[0] for a, b in run]
                    dve_last = waits(run[-1][1])[0]
                    run[0][0].sync_info = bass_rust.SyncInfo(
                        on_wait=[dve_last], on_update=[]
                    )
                    for ri, (a, b) in enumerate(run):
                        b.sync_info = bass_rust.SyncInfo(
                            on_wait=[pe_waits[ri]], on_update=upds(b)
                        )
                        if ri > 0:
                            drop.add(id(a))
                    p = q
                else:
                    p += 1
            bb.instructions = [x for x in kept if id(x) not in drop]
    return nc


_cache = {}


def _get_nc():
    if "nc" not in _cache:
        _cache["nc"] = _optimize_act_waits(_split_multi_waits(build_kernel()))
    return _cache["nc"]


def kernel(h, adj_c, W_query, W_key, W_val, W_out, trace=False):
    h = np.asarray(h, np.float32)
    adj = np.asarray(adj_c)
    hT = np.ascontiguousarray(
        h.transpose(0, 2, 1).astype(ml_dtypes.bfloat16)
    )  # [B, E, N] bf16
    adjT = np.ascontiguousarray(
        adj.transpose(0, 2, 1).astype(ml_dtypes.bfloat16)
    )  # [B, N(m), N(n)] bf16
    wq = np.ascontiguousarray(
        np.asarray(W_query, np.float32).transpose(1, 0, 2).reshape(E, H * KD)
    ).astype(ml_dtypes.bfloat16)
    wk = np.ascontiguousarray(
        np.asarray(W_key, np.float32).transpose(1, 0, 2).reshape(E, H * KD)
    ).astype(ml_dtypes.bfloat16)
    wv = np.ascontiguousarray(
        np.asarray(W_val, np.float32).transpose(1, 0, 2).reshape(E, H * KD)
    ).astype(ml_dtypes.bfloat16)
    # rows in slot order: group-0 heads (0,2,4,6) then group-1 (1,3,5,7),
    # matching the on-device headsTu/den packing
    wo = np.ascontiguousarray(
        np.asarray(W_out, np.float32)[[0, 2, 4, 6, 1, 3, 5, 7]].reshape(
            H * KD, E
        )
    ).astype(ml_dtypes.bfloat16)

    nc = _get_nc()
    in_maps = []
    for c in range(CORES):
        s = slice(c * BPC, (c + 1) * BPC)
        in_maps.append(
            {
                "ht": np.ascontiguousarray(hT[s]),
                "hn": np.ascontiguousarray(h[s]),
                "adjt": np.ascontiguousarray(adjT[s]),
                "wq": wq,
                "wk": wk,
                "wv": wv,
                "wo": wo,
            }
        )
    res = run_bass_kernel_spmd(nc, in_maps, core_ids=list(range(CORES)), trace=trace)
    out = np.concatenate([r["out"] for r in res.results], axis=0)
    if trace:
        return out, res
    return out


# revision 52
# speedup vs baseline: 1.1828x; 1.1828x over previous
"""Multi-head graph attention (GAT-style) Trainium2 Bass kernel.

Full-input contract: kernel(**inputs) takes the complete arrays, shards
batch-wise across 8 NeuronCores (2 batches each), and gathers the output.

Math per batch b, head h (KD=16 head dim):
  Q = h @ Wq_h, K = h @ Wk_h, V = h @ Wv_h            [N, 16]
  compatT[m, n] = (K Q^T)[m, n]                        [N, N] (transposed)
  p = exp(0.25 * compatT) * adjT                       (mask after exp; exact:
      masked entries are exactly 0, matching softmax(-inf) * adj)
  headsT[v, n] = (V'.T @ p)  with V' = [V | 1]         -> row 16 = denominator
  out[n, :] = sum_h (headsT_h / denom_h).T @ Wout_h + h[n, :]

The scalar engine's 128 exp instructions (~1.1us each over [128,1024] PSUM
tiles) are the hard roofline (~140us busy; ACT runs 1 elem/cycle/lane at
1.2GHz, no other engine has exp); everything else is scheduled to hide
under it:
  - hT / weights shipped bf16 from host; all-bf16 projections and PV.
  - heads regrouped as {0,2,4,6} (group 0) / {1,3,5,7} (group 1): heads
    0/2/4 are read by compat straight from the packed projection tile at
    partition bases 0/32/64 via tile_position rows (no shift DMAs on the
    first-compat critical path); the other five shift to a [16,5,N] tile
    during group 0. headsTu/den/Wout use slot order (Wout rows reordered
    host-side).
  - per-mc emission order [c0 c1 PV(prev-mc) c2 c3]: each compat reuses the
    PSUM buffer freed by the exp two slots earlier, and the PV quad (4
    heads column-packed via tile_position, which overlaps their streams)
    sits where the PE would otherwise idle.
  - V projection spread one chunk per mc-yield; batch b+1's load + QK
    projection emitted at batch-b group-1 start, its V chunks across
    group-1's yields; batch b's normalize/out-project deferred into batch
    b+1's group-0 stream so the reciprocal DMA chain is off the critical
    path.
  - DMA descriptors cost ~600ns each to issue and one descriptor rides one
    HW queue, so bulk loads go on the gpsimd DGE queue, hT split by column
    halves, adj split per m-chunk, group-end row unpacks split across
    sync+gpsimd.
  - denominator reciprocal on [128, 64] (n spread over 16 partitions via
    reshape DMA) because InstReciprocal costs free-size x 8 passes; the
    per-(head,n) reciprocal broadcast to all 16 v-rows is a single
    DRAM-bounce DMA with a stride-0 AP dimension.

Measured no-win dead ends: fp8 DoubleRow compat (HW streams the doubled
ifmap serially, and fp8 Q/K alone costs 3e-2 rel err), 1024-col matmuls
(s3d3_mm_num_elements), exp-pair [128,2048] tiles (PSUM bank budget),
custom-DVE exp (no float<->int convert; shifts return 0), splitting PV
quads (breaks the column-pack overlap), split-accumulated finish across
tile_position row bases (wrong results).
"""

import numpy as np
import ml_dtypes
from contextlib import ExitStack

import concourse.bass as bass
import concourse.mybir as mybir
import concourse.tile as tile
from concourse.bass_utils import run_bass_kernel_spmd

B, N, E, H, KD = 16, 1024, 128, 8, 16
CORES = 8
BPC = B // CORES  # batches per core
F32 = mybir.dt.float32
BF16 = mybir.dt.bfloat16
NT = 512  # PSUM bank = 512 fp32 -> max matmul out free dim
MC = N // 128  # number of 128-row chunks of m / n


def build_kernel():
    nc = bass.Bass()
    hT_d = nc.dram_tensor("ht", [BPC, E, N], BF16, kind="ExternalInput")
    h_d = nc.dram_tensor("hn", [BPC, N, E], F32, kind="ExternalInput")
    adjt_d = nc.dram_tensor("adjt", [BPC, N, N], BF16, kind="ExternalInput")
    wq_d = nc.dram_tensor("wq", [E, H * KD], BF16, kind="ExternalInput")
    wk_d = nc.dram_tensor("wk", [E, H * KD], BF16, kind="ExternalInput")
    wv_d = nc.dram_tensor("wv", [E, H * KD], BF16, kind="ExternalInput")
    wo_d = nc.dram_tensor("wo", [H * KD, E], BF16, kind="ExternalInput")
    out_d = nc.dram_tensor("out", [BPC, N, E], F32, kind="ExternalOutput")

    with ExitStack() as ctx:
        tc = ctx.enter_context(tile.TileContext(nc))
        consts = ctx.enter_context(tc.tile_pool(name="consts", bufs=1))
        io_pool = ctx.enter_context(tc.tile_pool(name="io", bufs=2))
        qk_pool = ctx.enter_context(tc.tile_pool(name="qk", bufs=2))
        v_pool = ctx.enter_context(tc.tile_pool(name="v", bufs=2))
        pt_pool = ctx.enter_context(tc.tile_pool(name="pt", bufs=8))
        p_pool = ctx.enter_context(tc.tile_pool(name="p", bufs=3))
        hd_pool = ctx.enter_context(tc.tile_pool(name="hd", bufs=2))
        ob_pool = ctx.enter_context(tc.tile_pool(name="ob", bufs=2))
        # PSUM: 8 banks of [128, 512]f32. compat 2x2 + PV 2 + misc 2 = 8.
        ps_c = ctx.enter_context(tc.tile_pool(name="ps_c", bufs=2, space="PSUM"))
        ps_h = ctx.enter_context(tc.tile_pool(name="ps_h", bufs=1, space="PSUM"))
        ps_m = ctx.enter_context(tc.tile_pool(name="ps_m", bufs=1, space="PSUM"))
        dram = ctx.enter_context(tc.tile_pool(name="dram", bufs=2, space="DRAM"))

        wq_sb = consts.tile([E, H * KD], BF16, tag="wq")
        wk_sb = consts.tile([E, H * KD], BF16, tag="wk")
        wv_sb = consts.tile([E, H * KD], BF16, tag="wv")
        wo_sb = consts.tile([H * KD, E], BF16, tag="wo")
        # weights on gpsimd so the sync queue's first descriptors are the
        # hT halves that gate the first projection matmul
        nc.gpsimd.dma_start(out=wq_sb, in_=wq_d[:, :])
        nc.gpsimd.dma_start(out=wk_sb, in_=wk_d[:, :])
        nc.gpsimd.dma_start(out=wv_sb, in_=wv_d[:, :])
        nc.gpsimd.dma_start(out=wo_sb, in_=wo_d[:, :])

        def load(b):
            # bulk loads go on the gpsimd DGE queue: the sync sequencer's
            # ~600ns per-DMA issue otherwise serializes ahead of the Q/K
            # shift DMAs that gate the first compat matmul
            # one descriptor = one HW queue, so split hT by column halves:
            # the first QK projection matmul only needs cols 0..511
            hT_sb = io_pool.tile([E, N], BF16, tag="ht")
            nc.sync.dma_start(out=hT_sb[:, 0:NT], in_=hT_d[b, :, 0:NT])
            nc.sync.dma_start(out=hT_sb[:, NT:N], in_=hT_d[b, :, NT:N])
            adjT_sb = io_pool.tile([128, MC, N], BF16, tag="adj")
            # split per m-chunk so mask(mc0) doesn't wait on the full 2MB
            for m in range(MC):
                nc.gpsimd.dma_start(
                    out=adjT_sb[:, m, :], in_=adjt_d[b, m * 128 : (m + 1) * 128, :]
                )
            h_sb = io_pool.tile([128, MC, E], F32, tag="hn")
            nc.gpsimd.dma_start(
                out=h_sb, in_=h_d[b].rearrange("(c p) e -> p c e", p=128)
            )
            return hT_sb, h_sb, adjT_sb

        def proj_qk(hT_sb):
            """Q/K projections (bf16). Group-0 heads {0,2,4} are read by
            compat directly from the packed tile at partition bases
            0/32/64 (legal tile_position rows) -> zero shift DMAs on the
            first-compat critical path. Heads {6,1,3,5,7} are shifted to a
            [16, 5, N] tile during group 0 (5 descriptors per tensor)."""
            packs = []
            for w_sb, tag in ((wq_sb, "q"), (wk_sb, "k")):
                ps = ps_m.tile([128, N], F32, tag="misc")
                for nt in range(N // NT):
                    nc.tensor.matmul(
                        out=ps[:, nt * NT : (nt + 1) * NT],
                        lhsT=w_sb,
                        rhs=hT_sb[:, nt * NT : (nt + 1) * NT],
                        start=True,
                        stop=True,
                    )
                packed = qk_pool.tile([128, N], BF16, tag=f"{tag}pk")
                nc.vector.tensor_copy(out=packed, in_=ps)
                per16 = qk_pool.tile([16, 5, N], BF16, tag=f"{tag}16")
                packs.append((packed, per16))
            # head 6 would need base partition 96 (only 0/32/64 are legal),
            # so it is shifted along with the odd heads
            for idx, hi in enumerate((6, 1, 3, 5, 7)):
                for packed, per16 in packs:
                    nc.sync.dma_start(
                        out=per16[:, idx, :],
                        in_=packed[hi * KD : (hi + 1) * KD, :],
                    )
            return packs

        def proj_v_chunk(b, m):
            """One V chunk: natural [m, h, 17] bf16, col 16 = ones."""
            st = state[b]
            v_ps = ps_m.tile([128, N], F32, tag="misc")
            nc.tensor.matmul(
                out=v_ps[:, 0 : H * KD],
                lhsT=st["hT"][:, m * 128 : (m + 1) * 128],
                rhs=wv_sb,
                start=True,
                stop=True,
            )
            vt = v_pool.tile([128, H, KD + 1], BF16, tag=f"v{m}")
            nc.vector.tensor_copy(
                out=vt[:, :, 0:KD],
                in_=v_ps[:, 0 : H * KD].rearrange("p (h k) -> p h k", k=KD),
            )
            nc.vector.memset(vt[:, :, KD : KD + 1], 1.0)
            st["v"][m] = vt

        state = [None] * BPC

        def start_batch(b):
            hT_sb, h_sb, adjT_sb = load(b)
            qpk, kpk = proj_qk(hT_sb)
            headsTu = hd_pool.tile([128, N], F32, tag="hTu", name="headsTu")
            den128 = hd_pool.tile([128, N // 16], F32, tag="den128", name="den128")
            state[b] = dict(
                hT=hT_sb, h=h_sb, adjT=adjT_sb, qpk=qpk, kpk=kpk,
                v=[None] * MC, headsTu=headsTu, den128=den128,
            )

        HEADS = ((0, 2, 4, 6), (1, 3, 5, 7))

        def attn_group(b, g):
            """Attention for heads HEADS[g] of batch b (slot = 4g+jj).
            Yields after each mc's emission so callers can interleave
            projection work."""
            st = state[b]
            adjT_sb = st["adjT"]
            hp4 = ps_h.tile([128, N], F32, tag="heads")  # slot j at rows 32j

            def emit_compat(m, jj):
                hi = HEADS[g][jj]
                if g == 0 and jj < 3:
                    # heads 0/2/4: direct from the packed tile at partition
                    # base hi*16 in {0,32,64} (legal tile_position rows)
                    k_ap = st["kpk"][0][hi * KD : (hi + 1) * KD, :]
                    q_ap = st["qpk"][0][hi * KD : (hi + 1) * KD, :]
                else:
                    idx = 0 if g == 0 else 1 + jj  # per16 slot
                    k_ap = st["kpk"][1][:, idx, :]
                    q_ap = st["qpk"][1][:, idx, :]
                c_ps = ps_c.tile([128, N], F32, tag="compat")
                for nt in range(N // NT):
                    nc.tensor.matmul(
                        out=c_ps[:, nt * NT : (nt + 1) * NT],
                        lhsT=k_ap[:, m * 128 : (m + 1) * 128],
                        rhs=q_ap[:, nt * NT : (nt + 1) * NT],
                        start=True,
                        stop=True,
                    )
                pT = pt_pool.tile([128, N], BF16, tag="pt")
                nc.scalar.activation(
                    out=pT,
                    in_=c_ps,
                    func=mybir.ActivationFunctionType.Exp,
                    scale=0.25,
                )
                pm = p_pool.tile([128, N], BF16, tag=f"pm{jj}")
                nc.vector.tensor_mul(pm, pT, adjT_sb[:, m, :])
                return pm

            def emit_pv(pms, m, jjs):
                vt = st["v"][m]
                for nt in range(N // NT):
                    for jj in jjs:
                        nc.tensor.matmul(
                            out=hp4[
                                32 * jj : 32 * jj + KD + 1,
                                nt * NT : (nt + 1) * NT,
                            ],
                            lhsT=vt[:, HEADS[g][jj], :],
                            rhs=pms[jj][:, nt * NT : (nt + 1) * NT],
                            start=(m == 0),
                            stop=(m == MC - 1),
                            tile_position=(0, 32 * jj),
                        )

            prev = None
            for m in range(MC):
                cur = [emit_compat(m, 0), emit_compat(m, 1)]
                if prev is not None:
                    emit_pv(prev, m - 1, (0, 1, 2, 3))
                cur += [emit_compat(m, 2), emit_compat(m, 3)]
                prev = cur
                yield m
            emit_pv(prev, MC - 1, (0, 1, 2, 3))
            # unpack heads + denominator rows: PSUM->SBUF copy on the idle
            # Pool engine (DMA can't read PSUM; DVE is near its budget),
            # then row DMAs on the tensor DGE queue (sync queue is the
            # startup hot path)
            hu4 = hd_pool.tile([128, N], F32, tag="huh")
            if b == BPC - 1 and g == 1:
                # tail-exposed copy: scalar engine is idle after the last exp
                nc.scalar.activation(
                    out=hu4, in_=hp4, func=mybir.ActivationFunctionType.Copy
                )
            else:
                nc.vector.tensor_copy(out=hu4, in_=hp4)
            # den rows first (they gate the reciprocal -> finish chain) on
            # sync; bulkier headsTu rows in parallel on gpsimd
            for jj in range(4):
                hi = g * 4 + jj  # slot index (wo is slot-reordered host-side)
                # [1, 1024] row -> [16, 64] block (reshape across partitions;
                # dst has real partition strides, so legal)
                nc.sync.dma_start(
                    out=st["den128"][hi * 16 : (hi + 1) * 16, :],
                    in_=hu4[32 * jj + KD : 32 * jj + KD + 1, :],
                )
            for jj in range(4):
                hi = g * 4 + jj
                nc.gpsimd.dma_start(
                    out=st["headsTu"][hi * KD : (hi + 1) * KD, :],
                    in_=hu4[32 * jj : 32 * jj + KD, :],
                )

        def recip_path(b):
            # den [8, N] lives as [128, N/16] (n spread over 16 partitions)
            # because InstReciprocal cost is free-size * 8 iterative passes
            st = state[b]
            recip128 = hd_pool.tile([128, N // 16], F32, tag="rec128")
            nc.vector.reciprocal(out=recip128, in_=st["den128"])
            rec_dram = dram.tile([8, N], F32, tag="recd")
            nc.gpsimd.dma_start(
                out=rec_dram.rearrange("h (a b) -> (h a) b", a=16),
                in_=recip128,
            )
            recip_bc = hd_pool.tile([128, N], F32, tag="recbc")
            bc = bass.AP(
                tensor=rec_dram.tensor,
                offset=rec_dram.offset,
                ap=[[N, H], [0, KD], [1, N]],
            )
            nc.gpsimd.dma_start(out=recip_bc, in_=bc)
            st["recip_bc"] = recip_bc

        def finish(b, last=False):
            st = state[b]
            headsTn = hd_pool.tile([128, N], BF16, tag="hnorm")
            nc.vector.tensor_mul(headsTn, st["headsTu"], st["recip_bc"])
            for cc in range(MC):
                # in the exposed tail (last batch) alternate PSUM pools so
                # the 8 out-projections pipeline instead of serializing on
                # one buffer; mid-stream ps_c belongs to the compat pipeline
                pool = ps_c if (last and cc % 2) else ps_m
                o_ps = pool.tile(
                    [128, N], F32, tag="compat" if pool is ps_c else "misc"
                )
                nc.tensor.matmul(
                    out=o_ps[:, 0:E],
                    lhsT=headsTn[:, cc * 128 : (cc + 1) * 128],
                    rhs=wo_sb,
                    start=True,
                    stop=True,
                )
                ob = ob_pool.tile([128, E], F32, tag="ob")
                nc.vector.tensor_add(ob, o_ps[:, 0:E], st["h"][:, cc, :])
                nc.sync.dma_start(
                    out=out_d[b, cc * 128 : (cc + 1) * 128, :], in_=ob
                )

        # ---- pipelined emission ----
        start_batch(0)
        pending = None
        for b in range(BPC):
            for g in range(2):
                for m in attn_group(b, g):
                    if g == 0:
                        # spread own V chunks under group-0's exp stream
                        # (batch 1's were already built during batch 0's g1)
                        if state[b]["v"][m] is None:
                            proj_v_chunk(b, m)
                        if m == 2 and pending is not None:
                            finish(pending)
                            pending = None
                    else:
                        if b + 1 < BPC:
                            if m == 0:
                                start_batch(b + 1)
                            proj_v_chunk(b + 1, m)
            recip_path(b)
            pending = b
        finish(pending, last=True)
    return nc


def _split_multi_waits(nc):
    """walrus codegen in this container allows only one sync-wait per
    instruction; hoist extra waits onto preceding same-engine nops."""
    import copy
    import bass_rust

    tmpl_nc = bass.Bass()
    tmpls = {}
    for en in ["vector", "scalar", "tensor", "gpsimd", "sync"]:
        ins = getattr(tmpl_nc, en).nop().ins
        tmpls[str(ins.engine)] = ins

    uid = [0]
    for fn in nc.m.functions:
        for bb in fn.blocks:
            out = []
            for ins in bb.instructions:
                si = ins.sync_info
                waits = list(si.on_wait) if si is not None else []
                if len(waits) > 1:
                    for w in waits[:-1]:
                        nop = copy.deepcopy(tmpls[str(ins.engine)])
                        uid[0] += 1
                        nop.name = f"I-splitw-{uid[0]}"
                        nop.sync_info = bass_rust.SyncInfo(
                            on_wait=[w], on_update=[]
                        )
                        out.append(nop)
                    ins.sync_info = bass_rust.SyncInfo(
                        on_wait=[waits[-1]], on_update=list(si.on_update)
                    )
                out.append(ins)
            bb.instructions = out
    return nc


def _optimize_act_waits(nc):
    """The ACT sequencer has no instruction lookahead, so each of its
    instructions costs ~100ns of decode+wait processing. Steady state per
    exp is [nop(wait ACT-self), nop(wait PE), exp(wait DVE)]:
    (1) drop ACT nops waiting on ACT's own semaphore — that WAW (exp i vs
        exp i-8 reusing a pT buffer) is already ordered by in-order
        same-engine execution;
    (2) in runs of [nop(PE), exp(DVE)] pairs, put the PE compat RAW wait
        (the tight one) on the exp itself and hoist one DVE wait — the
        strongest (last) of up to 4 pairs — in front of the run. The DVE
        wait guards pT-buffer reuse vs mask(i-8); with the 8-deep pT pool
        and masks trailing exps by ~1 slot it is satisfied ~4 exp periods
        before it is checked, so hoisting cannot stall."""
    import bass_rust

    def is_act(x):
        return "Activation" in str(x.engine)

    def waits(x):
        return list(x.sync_info.on_wait) if x.sync_info else []

    def upds(x):
        return list(x.sync_info.on_update) if x.sync_info else []

    for fn in nc.m.functions:
        for bb in fn.blocks:
            # pass 1: drop redundant ACT-self-wait nops
            kept = []
            for ins in bb.instructions:
                if (
                    is_act(ins)
                    and type(ins).__name__ == "InstNoOp"
                    and not upds(ins)
                ):
                    w = waits(ins)
                    if len(w) == 1 and str(w[0].ant_name).startswith(
                        "Activation"
                    ):
                        continue
                kept.append(ins)
            # pass 2: on the ACT-only subsequence (other engines' entries
            # interleave in the merged list), merge [nop(PE), exp(DVE)]
            # pairs in groups of 4: exp takes its nop's PE wait; the first
            # nop takes the last pair's DVE wait; the other nops are
            # deleted. Instructions are not reordered.
            act_idx = [k for k, x in enumerate(kept) if is_act(x)]
            drop = set()
            p = 0
            while p < len(act_idx):
                run = []
                q = p
                while q + 1 < len(act_idx) and len(run) < 4:
                    if act_idx[q + 1] != act_idx[q] + 1 and any(
                        is_act(kept[t])
                        for t in range(act_idx[q] + 1, act_idx[q + 1])
                    ):
                        break  # unreachable; adjacency in ACT order holds
                    a, b = kept[act_idx[q]], kept[act_idx[q + 1]]
                    if not (
                        type(a).__name__ == "InstNoOp"
                        and type(b).__name__ == "InstActivation"
                    ):
                        break
                    wa, wb = waits(a), waits(b)
                    if (
                        len(wa) != 1
                        or len(wb) != 1
                        or upds(a)
                        or not str(wa[0].ant_name).startswith("PE")
                        or not str(wb[0].ant_name).startswith("DVE")
                    ):
                        break
                    run.append((a, b))
                    q += 2
                if len(run) >= 2:
                    pe_waits = [waits(a)[0] for a, b in run]
                    dve_last = waits(run[-1][1])[0]
                    run[0][0].sync_info = bass_rust.SyncInfo(
                        on_wait=[dve_last], on_update=[]
                    )
                    for ri, (a, b) in enumerate(run):
                        b.sync_info = bass_rust.SyncInfo(
                            on_wait=[pe_waits[ri]], on_update=upds(b)
                        )
                        if ri > 0:
                            drop.add(id(a))
                    p = q
                else:
                    p += 1
            bb.instructions = [x for x in kept if id(x) not in drop]
    return nc


_cache = {}


def _get_nc():
    # NOTE: _optimize_act_waits (dropping the redundant ACT-self-wait nops
    # and batching the pT WAR waits 4:1) is semantically correct but
    # measured 44us SLOWER on hardware (284us vs 239us) — the nops are
    # near-free on the sequencer and the restructured waits disturb the
    # pipeline. Kept for reference; deliberately not applied.
    if "nc" not in _cache:
        _cache["nc"] = _split_multi_waits(build_kernel())
    return _cache["nc"]


def kernel(h, adj_c, W_query, W_key, W_val, W_out, trace=False):
    h = np.asarray(h, np.float32)
    adj = np.asarray(adj_c)
    hT = np.ascontiguousarray(
        h.transpose(0, 2, 1).astype(ml_dtypes.bfloat16)
    )  # [B, E, N] bf16
    adjT = np.ascontiguousarray(
        adj.transpose(0, 2, 1).astype(ml_dtypes.bfloat16)
    )  # [B, N(m), N(n)] bf16
    wq = np.ascontiguousarray(
        np.asarray(W_query, np.float32).transpose(1, 0, 2).reshape(E, H * KD)
    ).astype(ml_dtypes.bfloat16)
    wk = np.ascontiguousarray(
        np.asarray(W_key, np.float32).transpose(1, 0, 2).reshape(E, H * KD)
    ).astype(ml_dtypes.bfloat16)
    wv = np.ascontiguousarray(
        np.asarray(W_val, np.float32).transpose(1, 0, 2).reshape(E, H * KD)
    ).astype(ml_dtypes.bfloat16)
    # rows in slot order: group-0 heads (0,2,4,6) then group-1 (1,3,5,7),
    # matching the on-device headsTu/den packing
    wo = np.ascontiguousarray(
        np.asarray(W_out, np.float32)[[0, 2, 4, 6, 1, 3, 5, 7]].reshape(
            H * KD, E
        )
    ).astype(ml_dtypes.bfloat16)

    nc = _get_nc()
    in_maps = []
    for c in range(CORES):
        s = slice(c * BPC, (c + 1) * BPC)
        in_maps.append(
            {
                "ht": np.ascontiguousarray(hT[s]),
                "hn": np.ascontiguousarray(h[s]),
                "adjt": np.ascontiguousarray(adjT[s]),
                "wq": wq,
                "wk": wk,
                "wv": wv,
                "wo": wo,
            }
        )
    res = run_bass_kernel_spmd(nc, in_maps, core_ids=list(range(CORES)), trace=trace)
    out = np.concatenate([r["out"] for r in res.results], axis=0)
    if trace:
        return out, res
    return out
